# revision 6
# baseline (speedup 1.0000x reference)
"""Two-layer GCN forward (GCNConv -> relu -> GCNConv -> log_softmax) on 8
Trainium2 NeuronCores, single NEFF.

Design (pull-based aggregation at width 16):
  - Nodes are degree-sorted and assigned to cores so every core owns NP nodes
    with an identical degree profile (group g on every core holds the nodes of
    global rank window [1024g, 1024(g+1)) -- required because the SPMD padded
    CSR schedule K[g][chunk] is shared across cores).
  - The "message table" t = dis * h (16 f32 per node) is AllGathered compactly
    ([NTBL, 16] f32) then locally expanded to a 256B-strided layout
    ([NTBL, 64] f32, row i = node i) so the bulk SWDGE gather instruction
    (InstDMAGatherAnt, 64B payload, 256B row stride, int16 idx) can fetch
    per-edge rows. int16 limits one gather to 32k rows, so the table is split
    into 4 chunks of 2*NPL rows (chunk c = cores 2c,2c+1's slices); a host-side
    balancer assigns nodes to core-pairs to equalize each target's in-neighbor
    count per chunk, shrinking the padded-CSR K.
  - Per (group, chunk) the padded CSR has K[g][c] slots per target; gathers are
    issued in <=8192-token instructions round-robin over the 4 SWDGE queues
    (descriptor generation on the Q7 cores is the bottleneck; 4 queues run it
    in parallel). Pad slots point at a dummy node row which is always zero
    (dummy nodes get dis = 0). A strided DVE tensor_reduce sums the K slots.
  - norm factorizes: norm[e] = dis[src]*dis[dst], dis = rsqrt(deg+1), so
    out = dis * (gathered_sum + t_self) + bias per layer; layer 2 aggregates at
    width 16 and applies W2 afterwards (aggregation commutes with the output
    projection).

kernel(**inputs) takes full unsharded inputs, returns the full [N, 40] output.
"""

import sys

import numpy as np

try:
    import concourse.bass as bass
except ImportError:  # pragma: no cover
    sys.path.insert(0, "/opt/trn_rl_repo")
    import concourse.bass as bass

import concourse.bacc as bacc
import concourse.tile as tile
from concourse import mybir
from concourse.bass_utils import run_bass_kernel_spmd
from concourse.masks import make_identity

F32 = mybir.dt.float32
BF16 = mybir.dt.bfloat16
I16 = mybir.dt.int16

NCORES = 8
P = 128
NCHUNK = 4
MAX_TOK = 8192  # tokens per gather instruction (HW-validated bound)
MAX_COLS = MAX_TOK // P


def _raw_dma_gather(g, out_ap, in_ap, idxs_ap, num_idxs, queue_num):
    """dma_gather with 64B payload (elem_size=16 f32) at 256B row stride.

    Mirrors bass's dma_gather lowering minus its 256B elem_size assert (the
    ucode only requires the row *stride* and the base address to be 256B
    aligned; 64B payloads verified on HW)."""
    from concourse.bass import MemorySpace

    assert idxs_ap.dtype == I16
    assert in_ap.space == MemorySpace.DRAM
    assert out_ap.space == MemorySpace.SBUF
    assert in_ap.ap[-1][1] == 16 and out_ap.ap[-1][1] == 16
    assert in_ap.ap[0][0] == 64  # elem_step: 64 f32 = 256B
    _in_ap = g.lower_ap_dma(in_ap, for_custom_bir_dma=True)
    _idxs_ap = g.lower_ap(idxs_ap)
    _out_ap = g.lower_ap(out_ap)
    return g.add_instruction(
        mybir.InstDMAGatherAnt(
            name=g.bass.get_next_instruction_name(),
            ins=[*_in_ap, _idxs_ap, g.lower_val_access(g.to_reg(num_idxs))],
            outs=[_out_ap],
            transpose=False,
            num_idxs=num_idxs,
            elem_size=16,
            stride_bytes_256=1,
            gen_mode=0,
            single_packet=False,
            queue_num=queue_num,
            sbuf_tokens_per_rank=0,
            sbuf_free_dim_per_rank=0,
            sbuf_free_dim_pad_per_rank=0,
            sbuf_byte_offset=0,
        )
    )


# --------------------------------------------------------------------------
# Host planner
# --------------------------------------------------------------------------

def _balance_chunks(row, col, deg, order, N, W, npasses=3, seed=0):
    """Assign each node to one of 4 chunks (= core pairs), W//4 per W-rank
    window, minimizing per-(target, chunk) in-neighbor overflow vs the
    per-target quota ceil(deg/4). Returns chunk[node] in 0..3."""
    quota_t = np.ceil(deg / NCHUNK).astype(np.int32)
    rng = np.random.default_rng(seed)
    chunk = np.empty(N, dtype=np.int8)
    chunk[order] = (np.arange(N) % NCHUNK).astype(np.int8)

    k = np.zeros((N, NCHUNK), dtype=np.int32)
    np.add.at(k, (col, chunk[row]), 1)

    src_sort = np.argsort(row, kind="stable")
    counts = np.bincount(row, minlength=N)
    starts = np.concatenate([[0], np.cumsum(counts)])
    NW = -(-N // W)

    for _ in range(npasses):
        for w in range(NW):
            ranks = np.arange(w * W, min((w + 1) * W, N))
            nodes = order[ranks]
            lo = starts[nodes]
            hi = starts[nodes + 1]
            lens = hi - lo
            if lens.sum() == 0:
                continue
            idx_edges = np.concatenate(
                [src_sort[a:b] for a, b in zip(lo, hi)])
            esrc = row[idx_edges]
            etgt = col[idx_edges]
            np.add.at(k, (etgt, chunk[esrc]), -1)
            pen = np.maximum(
                0, k[etgt] - quota_t[etgt][:, None] + 1).astype(np.float32)
            node_of_edge = np.repeat(np.arange(len(nodes)), lens)
            score = np.zeros((len(nodes), NCHUNK), dtype=np.float32)
            np.add.at(score, node_of_edge, pen)
            score += rng.random(score.shape).astype(np.float32) * 1e-3
            q = np.full(NCHUNK, len(nodes) // NCHUNK, dtype=np.int64)
            q[: len(nodes) - q.sum()] += 1
            srt = np.argsort(score, axis=1)
            regret = (score[np.arange(len(nodes)), srt[:, 1]]
                      - score[np.arange(len(nodes)), srt[:, 0]])
            for i in np.argsort(-regret):
                for c in srt[i]:
                    if q[c] > 0:
                        chunk[nodes[i]] = c
                        q[c] -= 1
                        break
            np.add.at(k, (etgt, chunk[esrc]), 1)
    return chunk, k


def _plan(edge_index, N):
    row = np.asarray(edge_index[0], dtype=np.int64)
    col = np.asarray(edge_index[1], dtype=np.int64)
    E = row.size

    NP = N // NCORES               # real nodes per core
    G = -(-NP // P)                # groups per core
    NPL = G * P                    # padded nodes per core
    NTBL = NCORES * NPL            # table rows
    CHROWS = 2 * NPL               # rows per chunk (= core pair)
    W = NCORES * P                 # rank window feeding one group (1024)

    deg = np.bincount(col, minlength=N)
    order = np.argsort(-deg, kind="stable")

    chunk, k_tc = _balance_chunks(row, col, deg, order, N, W)

    # core assignment: within chunk c, nodes by rank alternate cores 2c/2c+1
    rank = np.empty(N, dtype=np.int64)
    rank[order] = np.arange(N)
    core = np.empty(N, dtype=np.int64)
    lr = np.empty(N, dtype=np.int64)
    for c in range(NCHUNK):
        nodes_c = np.nonzero(chunk == c)[0]
        o = np.argsort(rank[nodes_c], kind="stable")
        nodes_c = nodes_c[o]
        core[nodes_c] = 2 * c + (np.arange(nodes_c.size) % 2)
        lr[nodes_c] = np.arange(nodes_c.size) // 2

    assert lr.max() < NP, f"core overflow: {lr.max()} >= {NP}"
    tid = core * NPL + lr
    cidx = (tid % CHROWS).astype(np.int64)
    ZIDX = NP  # dummy row (lr = NP of the even core) -> always zero
    assert NP < NPL, "need at least one dummy row per core"

    # K schedule: K[g][c] = max in-neighbor-count-from-chunk-c over targets in
    # group g (across all cores; group g <-> rank window g)
    g_of_node = lr // P
    K = np.zeros((G, NCHUNK), dtype=np.int64)
    for c in range(NCHUNK):
        kk = np.zeros(G, dtype=np.int64)
        np.maximum.at(kk, g_of_node, k_tc[:, c])
        K[:, c] = kk
    K = np.maximum(K, 0)

    # instruction packing: per chunk, consecutive groups with sum(K) <= MAX_COLS
    instrs = []  # (c, g0, n_groups, cols, col0) ; col0 = within-chunk col offset
    colstart = np.zeros((G, NCHUNK), dtype=np.int64)
    tokens_c = np.zeros(NCHUNK, dtype=np.int64)
    for c in range(NCHUNK):
        colstart[:, c] = np.concatenate([[0], np.cumsum(K[:, c])[:-1]])
        tokens_c[c] = K[:, c].sum() * P
        g0 = 0
        while g0 < G:
            cols = 0
            g1 = g0
            while g1 < G and cols + K[g1, c] <= MAX_COLS:
                cols += K[g1, c]
                g1 += 1
            if cols == 0:  # single group exceeding MAX_COLS (can't happen: K<=64)
                raise AssertionError("group K exceeds MAX_COLS")
            instrs.append((c, g0, g1 - g0, int(cols), int(colstart[g0, c])))
            g0 = g1

    # K-runs per instruction for the reduce stage
    runs = []  # per instr: list of (local_col0, n_groups, K, g0)
    for (c, g0, ng, cols, col0) in instrs:
        rr = []
        i = g0
        while i < g0 + ng:
            j = i
            while j < g0 + ng and K[j, c] == K[i, c]:
                j += 1
            if K[i, c] > 0:
                rr.append((int(colstart[i, c] - col0), j - i, int(K[i, c]), i))
            i = j
        runs.append(rr)

    # ---- per-core slot tables (idx arrays) ----
    # edge slot: target (m, g, p), source chunk c, k-th occurrence
    m_t = core[col]
    g_t = g_of_node[col]
    p_t = (lr % P)[col]
    c_s = chunk[row].astype(np.int64)
    cid_s = cidx[row]

    # sort edges by (m_t, c_s, g_t, p_t)
    key_order = np.lexsort((p_t, g_t, c_s, m_t))
    sm, sc, sg, sp, scid = (m_t[key_order], c_s[key_order], g_t[key_order],
                            p_t[key_order], cid_s[key_order])
    # k within each (m, c, g, p) run
    cell = ((sm * NCHUNK + sc) * G + sg) * P + sp
    cell_counts = np.bincount(cell, minlength=NCORES * NCHUNK * G * P)
    cell_starts = np.concatenate([[0], np.cumsum(cell_counts)[:-1]])
    kk = np.arange(E, dtype=np.int64) - cell_starts[cell]
    assert (kk < K[sg, sc]).all(), "slot overflow: K schedule too small"

    # token index within (m, c): j = (colstart[g, c] + k) * P + p
    j_tok = (colstart[sg, sc] + kk) * P + sp

    TOKC = tokens_c  # per-chunk token count (same for all cores)
    SROW = TOKC // 16
    idx_arrays = []
    for m in range(NCORES):
        arrs = []
        for c in range(NCHUNK):
            arr = np.full(TOKC[c], ZIDX, dtype=np.int16)
            sel = (sm == m) & (sc == c)
            arr[j_tok[sel]] = scid[sel].astype(np.int16)
            # wrapped layout: idx16[j%16, j//16]
            arrs.append(np.ascontiguousarray(
                arr.reshape(-1, 16).T))  # [16, TOKC/16]
        idx_arrays.append(np.concatenate(arrs, axis=1))  # [16, sum(SROW)]

    # idx column offset of each instruction within the concatenated idx array
    chunk_col0 = np.concatenate([[0], np.cumsum(SROW)[:-1]])
    instr_idx_off = [int(chunk_col0[c] + col0 * (P // 16))
                     for (c, g0, ng, cols, col0) in instrs]

    # per-core node list in (g, p) order: node at lr
    nodes_of_core = np.full((NCORES, NPL), -1, dtype=np.int64)
    for m in range(NCORES):
        sel = np.nonzero(core == m)[0]
        nodes_of_core[m, lr[sel]] = sel

    # deg upload [P, G]: deg[p, g] ; dummies inf
    degp = np.full((NCORES, P, G), np.inf, dtype=np.float32)
    for m in range(NCORES):
        sel = nodes_of_core[m] >= 0
        lrs = np.nonzero(sel)[0]
        degp[m, lrs % P, lrs // P] = deg[nodes_of_core[m][lrs]] + 1.0

    return dict(
        NP=NP, G=G, NPL=NPL, NTBL=NTBL, CHROWS=CHROWS,
        K=K, instrs=instrs, runs=runs, instr_idx_off=instr_idx_off,
        idx_arrays=idx_arrays, idx_cols=int(SROW.sum()),
        nodes_of_core=nodes_of_core, degp=degp, tokens_c=TOKC,
    )


# --------------------------------------------------------------------------
# Device program
# --------------------------------------------------------------------------

def _build_program(plan, IN_CH, HID, OUT_CH):
    G, NPL, NTBL, CHROWS = plan["G"], plan["NPL"], plan["NTBL"], plan["CHROWS"]
    instrs, runs, idx_off = plan["instrs"], plan["runs"], plan["instr_idx_off"]
    NCH_IN = IN_CH // P
    IDXC = plan["idx_cols"]

    nc = bacc.Bacc(None, num_devices=NCORES, num_swdge_queues=4)

    xT_d = nc.dram_tensor("xT", [IN_CH, NPL], BF16, kind="ExternalInput")
    deg_d = nc.dram_tensor("deg", [P, G], F32, kind="ExternalInput")
    idx_d = nc.dram_tensor("idx", [16, IDXC], I16, kind="ExternalInput")
    w1_d = nc.dram_tensor("w1", [P, NCH_IN * HID], BF16, kind="ExternalInput")
    b1_d = nc.dram_tensor("b1", [1, HID], F32, kind="ExternalInput")
    w2_d = nc.dram_tensor("w2", [HID, OUT_CH], F32, kind="ExternalInput")
    b2_d = nc.dram_tensor("b2", [1, OUT_CH], F32, kind="ExternalInput")
    out_d = nc.dram_tensor("out", [P, G * OUT_CH], F32, kind="ExternalOutput")

    groups_all = [list(range(NCORES))]

    with tile.TileContext(nc) as tc:
        with (
            tc.tile_pool(name="const", bufs=1) as const,
            tc.tile_pool(name="persist", bufs=1) as persist,
            tc.tile_pool(name="idxp", bufs=4) as idxp,
            tc.tile_pool(name="gbp", bufs=4) as gbp,
            tc.tile_pool(name="vts", bufs=3) as vtsp,
            tc.tile_pool(name="dram", bufs=1, space="DRAM") as dram,
            tc.tile_pool(name="psH", bufs=4, space="PSUM") as psH,
            tc.tile_pool(name="psT", bufs=2, space="PSUM") as psT,
            tc.tile_pool(name="psO", bufs=2, space="PSUM") as psO,
        ):
            # ---- constants ----
            ident = const.tile([P, P], F32)
            make_identity(nc, ident[:])
            w1st = vtsp.tile([P, NCH_IN * HID], BF16, tag="vts")
            nc.sync.dma_start(w1st[:], w1_d[:])
            w1sb = const.tile([P, NCH_IN, HID], BF16)
            nc.vector.tensor_copy(
                w1sb[:], w1st[:].rearrange("k (c f) -> k c f", f=HID))
            w2st = vtsp.tile([HID, OUT_CH], F32, tag="vts2")
            nc.sync.dma_start(w2st[:], w2_d[:])
            w2sb = const.tile([HID, OUT_CH], F32)
            nc.vector.tensor_copy(w2sb[:], w2st[:])
            b1sb = const.tile([P, HID], F32)
            nc.sync.dma_start(b1sb[:], b1_d[:].to_broadcast([P, HID]))
            b2sb = const.tile([P, OUT_CH], F32)
            nc.sync.dma_start(b2sb[:], b2_d[:].to_broadcast([P, OUT_CH]))
            degsb = const.tile([P, G], F32)
            nc.sync.dma_start(degsb[:], deg_d[:])
            dis = const.tile([P, G], F32)
            nc.vector.reciprocal(dis[:], degsb[:])
            nc.scalar.activation(dis[:], dis[:],
                                 mybir.ActivationFunctionType.Sqrt)

            def dis_bc(F):
                return (dis[:].rearrange("p (g u) -> p g u", u=1)
                        .to_broadcast([P, G, F]))

            # ---- stage A: H = x @ W1  (lhsT = xT chunks) ----
            H = persist.tile([P, G * HID], F32, tag="H")
            with tc.tile_pool(name="xtp", bufs=1) as xtp:
                xts = []
                touch = xtp.tile([P, NCH_IN], BF16, tag="touch",
                                 name="touch")
                for c in range(NCH_IN):
                    xt_c = xtp.tile([P, NPL], BF16, tag=f"xt{c}",
                                    name=f"xt{c}")
                    nc.sync.dma_start(
                        xt_c[:], xT_d[c * P:(c + 1) * P, :])
                    # DVE touch: PE matmuls then wait on the DVE lane only
                    nc.vector.tensor_copy(touch[:, c:c + 1], xt_c[:, 0:1])
                    xts.append(xt_c)
                for g in range(G):
                    hps = psH.tile([P, HID], F32, tag="hps")
                    for c in range(NCH_IN):
                        nc.tensor.matmul(
                            hps[:],
                            lhsT=xts[c][:, g * P:(g + 1) * P],
                            rhs=w1sb[:, c, :],
                            start=(c == 0),
                            stop=(c == NCH_IN - 1),
                        )
                    nc.vector.tensor_copy(H[:, g * HID:(g + 1) * HID], hps[:])

            # ---- per-layer helpers ----
            def publish(tsrc, tag):
                """tsrc [P, G*HID] -> compact slice -> AllGather -> expanded
                256B-strided table. Returns the padded table dram tile."""
                compact_slice = dram.tile([NPL, HID], F32,
                                          tag=f"cs_{tag}", name=f"cs_{tag}")
                # node lr = g*128+p  ->  compact row lr
                nc.sync.dma_start(
                    compact_slice[:].rearrange("(g p) f -> p g f", p=P),
                    tsrc[:].rearrange("p (g f) -> p g f", f=HID),
                )
                compact = dram.tile([NTBL, HID], F32, tag=f"cf_{tag}",
                                    name=f"cf_{tag}", addr_space="Shared")
                nc.gpsimd.collective_compute(
                    "AllGather",
                    mybir.AluOpType.bypass,
                    ins=[compact_slice[:].opt()],
                    outs=[compact[:].opt()],
                    replica_groups=groups_all,
                )
                padded = dram.tile([NTBL, 64], F32, tag=f"pt_{tag}",
                                   name=f"pt_{tag}")
                for c in range(NCHUNK):
                    nc.sync.dma_start(
                        padded[c * CHROWS:(c + 1) * CHROWS, 0:HID],
                        compact[c * CHROWS:(c + 1) * CHROWS, :],
                    )
                return padded

            def gather_layer(padded, tag):
                """Run the padded-CSR gathers + reduces. Returns agg
                [P, G*HID] f32 (sum over in-neighbors of t-rows)."""
                partials = persist.tile([P, NCHUNK * G * HID], F32,
                                        tag="partials", name="partials")
                nc.vector.memset(partials[:], 0.0)
                for i, (c, g0, ng, cols, col0) in enumerate(instrs):
                    S = cols * (P // 16)
                    idx_t = idxp.tile([P, MAX_COLS * (P // 16)], I16,
                                      tag="idx")
                    for kgrp in range(8):
                        nc.sync.dma_start(
                            idx_t[16 * kgrp:16 * (kgrp + 1), 0:S],
                            idx_d[:, idx_off[i]:idx_off[i] + S],
                        )
                    itouch = idxp.tile([P, 1], I16, tag="itouch")
                    nc.vector.tensor_copy(itouch[:], idx_t[:, 0:1])
                    gb = gbp.tile([P, MAX_COLS, HID], F32, tag="gb")
                    _raw_dma_gather(
                        nc.gpsimd,
                        gb[:, 0:cols, :],
                        padded[c * CHROWS:(c + 1) * CHROWS, 0:HID],
                        idx_t[:, 0:S],
                        cols * P,
                        queue_num=i % 4,
                    )
                    for (lc0, nng, kv, gg0) in runs[i]:
                        nc.vector.tensor_reduce(
                            out=partials[:].rearrange(
                                "p (cg f) -> p cg f", f=HID)[
                                :, c * G + gg0: c * G + gg0 + nng, :],
                            in_=gb[:, lc0:lc0 + nng * kv, :].rearrange(
                                "p (n k) f -> p n f k", k=kv),
                            axis=mybir.AxisListType.X,
                            op=mybir.AluOpType.add,
                        )
                agg = persist.tile([P, G * HID], F32, tag="agg",
                                   name="agg")
                nc.vector.tensor_reduce(
                    out=agg[:].rearrange("p (g f) -> p g f", f=HID),
                    in_=partials[:].rearrange("p (c g f) -> p g f c",
                                              c=NCHUNK, f=HID),
                    axis=mybir.AxisListType.X,
                    op=mybir.AluOpType.add,
                )
                return agg

            # ---- layer 1 ----
            t1 = persist.tile([P, G * HID], F32, tag="t1")
            nc.vector.tensor_tensor(
                out=t1[:].rearrange("p (g f) -> p g f", f=HID),
                in0=H[:].rearrange("p (g f) -> p g f", f=HID),
                in1=dis_bc(HID), op=mybir.AluOpType.mult)
            tbl1 = publish(t1, "t1")
            agg1 = gather_layer(tbl1, "l1")

            # out1 = dis*(agg1 + t1) + b1 ; h1 = relu ; t2 = dis*h1
            nc.vector.tensor_tensor(out=agg1[:], in0=agg1[:], in1=t1[:],
                                    op=mybir.AluOpType.add)
            nc.vector.tensor_tensor(
                out=agg1[:].rearrange("p (g f) -> p g f", f=HID),
                in0=agg1[:].rearrange("p (g f) -> p g f", f=HID),
                in1=dis_bc(HID), op=mybir.AluOpType.mult)
            nc.vector.tensor_tensor(
                out=agg1[:].rearrange("p (g f) -> p g f", f=HID),
                in0=agg1[:].rearrange("p (g f) -> p g f", f=HID),
                in1=b1sb[:].rearrange("p (u f) -> p u f", u=1)
                    .to_broadcast([P, G, HID]),
                op=mybir.AluOpType.add)
            nc.scalar.activation(agg1[:], agg1[:],
                                 mybir.ActivationFunctionType.Relu)
            t2 = persist.tile([P, G * HID], F32, tag="t2")
            nc.vector.tensor_tensor(
                out=t2[:].rearrange("p (g f) -> p g f", f=HID),
                in0=agg1[:].rearrange("p (g f) -> p g f", f=HID),
                in1=dis_bc(HID), op=mybir.AluOpType.mult)

            # ---- layer 2 ----
            tbl2 = publish(t2, "t2")
            agg2 = gather_layer(tbl2, "l2")

            nc.vector.tensor_tensor(out=agg2[:], in0=agg2[:], in1=t2[:],
                                    op=mybir.AluOpType.add)
            nc.vector.tensor_tensor(
                out=agg2[:].rearrange("p (g f) -> p g f", f=HID),
                in0=agg2[:].rearrange("p (g f) -> p g f", f=HID),
                in1=dis_bc(HID), op=mybir.AluOpType.mult)
            V = agg2

            # ---- O = V @ W2 + b2, log_softmax ----
            warm = psT.tile([P, P], F32, tag="pt")
            nc.tensor.transpose(warm[:], ident[:], ident[:])
            O = persist.tile([P, G * OUT_CH], F32, tag="O")
            for g in range(G):
                vt_ps = psT.tile([P, P], F32, tag="pt")
                nc.tensor.transpose(
                    vt_ps[:HID, :], V[:, g * HID:(g + 1) * HID], ident[:])
                vt_sb = vtsp.tile([HID, P], F32, tag="vts3")
                nc.vector.tensor_copy(vt_sb[:], vt_ps[:HID, :])
                ops = psO.tile([P, OUT_CH], F32, tag="ops")
                nc.tensor.matmul(ops[:], lhsT=vt_sb[:], rhs=w2sb[:],
                                 start=True, stop=True)
                nc.vector.tensor_copy(O[:, g * OUT_CH:(g + 1) * OUT_CH],
                                      ops[:])

            O3 = O[:].rearrange("p (g f) -> p g f", f=OUT_CH)
            nc.vector.tensor_tensor(
                out=O3, in0=O3,
                in1=b2sb[:].rearrange("p (u f) -> p u f", u=1)
                    .to_broadcast([P, G, OUT_CH]),
                op=mybir.AluOpType.add)
            mx = const.tile([P, G], F32)
            nc.vector.tensor_reduce(out=mx[:], in_=O3,
                                    axis=mybir.AxisListType.X,
                                    op=mybir.AluOpType.max)
            nc.vector.tensor_tensor(
                out=O3, in0=O3,
                in1=mx[:].rearrange("p (g u) -> p g u", u=1)
                    .to_broadcast([P, G, OUT_CH]),
                op=mybir.AluOpType.subtract)
            nc.scalar.activation(O[:], O[:],
                                 mybir.ActivationFunctionType.Exp)
            sm = const.tile([P, G], F32)
            nc.vector.tensor_reduce(
                out=sm[:],
                in_=O[:].rearrange("p (g f) -> p g f", f=OUT_CH),
                axis=mybir.AxisListType.X, op=mybir.AluOpType.add)
            nc.scalar.activation(sm[:], sm[:],
                                 mybir.ActivationFunctionType.Ln)
            # back to logits: ln(exp(x)) = x to ~1e-7; then subtract ln(sum)
            nc.scalar.activation(O[:], O[:],
                                 mybir.ActivationFunctionType.Ln)
            nc.vector.tensor_tensor(
                out=O3, in0=O3,
                in1=sm[:].rearrange("p (g u) -> p g u", u=1)
                    .to_broadcast([P, G, OUT_CH]),
                op=mybir.AluOpType.subtract)
            nc.sync.dma_start(out_d[:], O[:])

    nc.finalize()
    return nc


# --------------------------------------------------------------------------
# Entry point
# --------------------------------------------------------------------------

def kernel(x, W1, b1, W2, b2, edge_index, _trace=False):
    import ml_dtypes

    x = np.asarray(x, dtype=np.float32)
    W1 = np.asarray(W1, dtype=np.float32)
    b1 = np.asarray(b1, dtype=np.float32)
    W2 = np.asarray(W2, dtype=np.float32)
    b2 = np.asarray(b2, dtype=np.float32)
    edge_index = np.asarray(edge_index)

    N, IN_CH = x.shape
    HID = W1.shape[1]
    OUT_CH = W2.shape[1]

    plan = _plan(edge_index, N)
    kernel._plan = plan
    G, NPL = plan["G"], plan["NPL"]
    NCH_IN = IN_CH // P

    nc = _build_program(plan, IN_CH, HID, OUT_CH)

    # per-core inputs
    w1p = np.ascontiguousarray(
        W1.reshape(NCH_IN, P, HID).transpose(1, 0, 2).reshape(P, NCH_IN * HID)
    ).astype(ml_dtypes.bfloat16)
    maps = []
    for m in range(NCORES):
        nodes = plan["nodes_of_core"][m]
        xTm = np.zeros((IN_CH, NPL), dtype=ml_dtypes.bfloat16)
        sel = nodes >= 0
        xTm[:, sel] = x[nodes[sel]].T.astype(ml_dtypes.bfloat16)
        maps.append(dict(
            xT=xTm,
            deg=np.ascontiguousarray(plan["degp"][m]),
            idx=np.ascontiguousarray(plan["idx_arrays"][m]),
            w1=w1p,
            b1=b1.reshape(1, HID),
            w2=W2,
            b2=b2.reshape(1, OUT_CH),
        ))

    r = run_bass_kernel_spmd(nc, maps, core_ids=list(range(NCORES)),
                             trace=_trace)
    kernel._exec_ns = r.exec_time_ns or 0

    out = np.empty((N, OUT_CH), dtype=np.float32)
    for m in range(NCORES):
        Om = np.asarray(r.results[m]["out"]).reshape(P, G, OUT_CH)
        nodes = plan["nodes_of_core"][m]
        sel = nodes >= 0
        lrs = np.nonzero(sel)[0]
        out[nodes[lrs]] = Om[lrs % P, lrs // P, :]
    return out


# revision 7
# speedup vs baseline: 1.3404x; 1.3404x over previous
"""Two-layer GCN forward (GCNConv -> relu -> GCNConv -> log_softmax) on 8
Trainium2 NeuronCores, single NEFF.

Design (pull-based aggregation at width 16):
  - Nodes are degree-sorted and assigned to cores so every core owns NP nodes
    with an identical degree profile (group g on every core holds the nodes of
    global rank window [1024g, 1024(g+1)) -- required because the SPMD padded
    CSR schedule K[g][chunk] is shared across cores).
  - The "message table" t = dis * h (16 f32 per node) is AllGathered compactly
    ([NTBL, 16] f32) then locally expanded to a 256B-strided layout
    ([NTBL, 64] f32, row i = node i) so the bulk SWDGE gather instruction
    (InstDMAGatherAnt, 64B payload, 256B row stride, int16 idx) can fetch
    per-edge rows. int16 limits one gather to 32k rows, so the table is split
    into 4 chunks of 2*NPL rows (chunk c = cores 2c,2c+1's slices); a host-side
    balancer assigns nodes to core-pairs to equalize each target's in-neighbor
    count per chunk, shrinking the padded-CSR K.
  - Per (group, chunk) the padded CSR has K[g][c] slots per target; gathers are
    issued in <=8192-token instructions round-robin over the 4 SWDGE queues
    (descriptor generation on the Q7 cores is the bottleneck; 4 queues run it
    in parallel). Pad slots point at a dummy node row which is always zero
    (dummy nodes get dis = 0). A strided DVE tensor_reduce sums the K slots.
  - norm factorizes: norm[e] = dis[src]*dis[dst], dis = rsqrt(deg+1), so
    out = dis * (gathered_sum + t_self) + bias per layer; layer 2 aggregates at
    width 16 and applies W2 afterwards (aggregation commutes with the output
    projection).

kernel(**inputs) takes full unsharded inputs, returns the full [N, 40] output.
"""

import sys

import numpy as np

try:
    import concourse.bass as bass
except ImportError:  # pragma: no cover
    sys.path.insert(0, "/opt/trn_rl_repo")
    import concourse.bass as bass

import concourse.bacc as bacc
import concourse.tile as tile
from concourse import mybir
from concourse.bass_utils import run_bass_kernel_spmd
from concourse.masks import make_identity

F32 = mybir.dt.float32
BF16 = mybir.dt.bfloat16
I16 = mybir.dt.int16

NCORES = 8
P = 128
NCHUNK = 4
MAX_TOK = 8192  # tokens per gather instruction (HW-validated bound)
MAX_COLS = MAX_TOK // P


def _raw_dma_gather(g, out_ap, in_ap, idxs_ap, num_idxs, queue_num):
    """dma_gather with 64B payload (elem_size=16 f32) at 256B row stride.

    Mirrors bass's dma_gather lowering minus its 256B elem_size assert (the
    ucode only requires the row *stride* and the base address to be 256B
    aligned; 64B payloads verified on HW)."""
    from concourse.bass import MemorySpace

    assert idxs_ap.dtype == I16
    assert in_ap.space == MemorySpace.DRAM
    assert out_ap.space == MemorySpace.SBUF
    assert in_ap.ap[-1][1] == 16 and out_ap.ap[-1][1] == 16
    assert in_ap.ap[0][0] == 64  # elem_step: 64 f32 = 256B
    _in_ap = g.lower_ap_dma(in_ap, for_custom_bir_dma=True)
    _idxs_ap = g.lower_ap(idxs_ap)
    _out_ap = g.lower_ap(out_ap)
    return g.add_instruction(
        mybir.InstDMAGatherAnt(
            name=g.bass.get_next_instruction_name(),
            ins=[*_in_ap, _idxs_ap, g.lower_val_access(g.to_reg(num_idxs))],
            outs=[_out_ap],
            transpose=False,
            num_idxs=num_idxs,
            elem_size=16,
            stride_bytes_256=1,
            gen_mode=0,
            single_packet=False,
            queue_num=queue_num,
            sbuf_tokens_per_rank=0,
            sbuf_free_dim_per_rank=0,
            sbuf_free_dim_pad_per_rank=0,
            sbuf_byte_offset=0,
        )
    )


# --------------------------------------------------------------------------
# Host planner
# --------------------------------------------------------------------------

def _balance_chunks(row, col, deg, order, N, W, npasses=3, seed=0):
    """Assign each node to one of 4 chunks (= core pairs), W//4 per W-rank
    window, minimizing per-(target, chunk) in-neighbor overflow vs the
    per-target quota ceil(deg/4). Returns chunk[node] in 0..3."""
    quota_t = np.ceil(deg / NCHUNK).astype(np.int32)
    rng = np.random.default_rng(seed)
    chunk = np.empty(N, dtype=np.int8)
    chunk[order] = (np.arange(N) % NCHUNK).astype(np.int8)

    k = np.zeros((N, NCHUNK), dtype=np.int32)
    np.add.at(k, (col, chunk[row]), 1)

    src_sort = np.argsort(row, kind="stable")
    counts = np.bincount(row, minlength=N)
    starts = np.concatenate([[0], np.cumsum(counts)])
    NW = -(-N // W)

    for _ in range(npasses):
        for w in range(NW):
            ranks = np.arange(w * W, min((w + 1) * W, N))
            nodes = order[ranks]
            lo = starts[nodes]
            hi = starts[nodes + 1]
            lens = hi - lo
            if lens.sum() == 0:
                continue
            idx_edges = np.concatenate(
                [src_sort[a:b] for a, b in zip(lo, hi)])
            esrc = row[idx_edges]
            etgt = col[idx_edges]
            np.add.at(k, (etgt, chunk[esrc]), -1)
            pen = np.maximum(
                0, k[etgt] - quota_t[etgt][:, None] + 1).astype(np.float32)
            node_of_edge = np.repeat(np.arange(len(nodes)), lens)
            score = np.zeros((len(nodes), NCHUNK), dtype=np.float32)
            np.add.at(score, node_of_edge, pen)
            score += rng.random(score.shape).astype(np.float32) * 1e-3
            q = np.full(NCHUNK, len(nodes) // NCHUNK, dtype=np.int64)
            q[: len(nodes) - q.sum()] += 1
            srt = np.argsort(score, axis=1)
            regret = (score[np.arange(len(nodes)), srt[:, 1]]
                      - score[np.arange(len(nodes)), srt[:, 0]])
            for i in np.argsort(-regret):
                for c in srt[i]:
                    if q[c] > 0:
                        chunk[nodes[i]] = c
                        q[c] -= 1
                        break
            np.add.at(k, (etgt, chunk[esrc]), 1)
    return chunk, k


def _plan(edge_index, N):
    row = np.asarray(edge_index[0], dtype=np.int64)
    col = np.asarray(edge_index[1], dtype=np.int64)
    E = row.size

    NP = N // NCORES               # real nodes per core
    G = -(-NP // P)                # groups per core
    NPL = G * P                    # padded nodes per core
    NTBL = NCORES * NPL            # table rows
    CHROWS = 2 * NPL               # rows per chunk (= core pair)
    W = NCORES * P                 # rank window feeding one group (1024)

    deg = np.bincount(col, minlength=N)
    order = np.argsort(-deg, kind="stable")

    chunk, k_tc = _balance_chunks(row, col, deg, order, N, W)

    # core assignment: within chunk c, nodes by rank alternate cores 2c/2c+1
    rank = np.empty(N, dtype=np.int64)
    rank[order] = np.arange(N)
    core = np.empty(N, dtype=np.int64)
    lr = np.empty(N, dtype=np.int64)
    for c in range(NCHUNK):
        nodes_c = np.nonzero(chunk == c)[0]
        o = np.argsort(rank[nodes_c], kind="stable")
        nodes_c = nodes_c[o]
        core[nodes_c] = 2 * c + (np.arange(nodes_c.size) % 2)
        lr[nodes_c] = np.arange(nodes_c.size) // 2

    assert lr.max() < NP, f"core overflow: {lr.max()} >= {NP}"
    tid = core * NPL + lr
    cidx = (tid % CHROWS).astype(np.int64)
    ZIDX = NP  # dummy row (lr = NP of the even core) -> always zero
    assert NP < NPL, "need at least one dummy row per core"

    # K schedule: K[g][c] = max in-neighbor-count-from-chunk-c over targets in
    # group g (across all cores; group g <-> rank window g)
    g_of_node = lr // P
    K = np.zeros((G, NCHUNK), dtype=np.int64)
    for c in range(NCHUNK):
        kk = np.zeros(G, dtype=np.int64)
        np.maximum.at(kk, g_of_node, k_tc[:, c])
        K[:, c] = kk
    K = np.maximum(K, 0)

    # instruction packing: per chunk, consecutive groups with sum(K) <= MAX_COLS
    instrs = []  # (c, g0, n_groups, cols, col0) ; col0 = within-chunk col offset
    colstart = np.zeros((G, NCHUNK), dtype=np.int64)
    tokens_c = np.zeros(NCHUNK, dtype=np.int64)
    for c in range(NCHUNK):
        colstart[:, c] = np.concatenate([[0], np.cumsum(K[:, c])[:-1]])
        tokens_c[c] = K[:, c].sum() * P
        g0 = 0
        while g0 < G:
            cols = 0
            g1 = g0
            while g1 < G and cols + K[g1, c] <= MAX_COLS:
                cols += K[g1, c]
                g1 += 1
            if cols == 0:  # single group exceeding MAX_COLS (can't happen: K<=64)
                raise AssertionError("group K exceeds MAX_COLS")
            instrs.append((c, g0, g1 - g0, int(cols), int(colstart[g0, c])))
            g0 = g1

    # K-runs per instruction for the reduce stage
    runs = []  # per instr: list of (local_col0, n_groups, K, g0)
    for (c, g0, ng, cols, col0) in instrs:
        rr = []
        i = g0
        while i < g0 + ng:
            j = i
            while j < g0 + ng and K[j, c] == K[i, c]:
                j += 1
            if K[i, c] > 0:
                rr.append((int(colstart[i, c] - col0), j - i, int(K[i, c]), i))
            i = j
        runs.append(rr)

    # ---- per-core slot tables (idx arrays) ----
    # edge slot: target (m, g, p), source chunk c, k-th occurrence
    m_t = core[col]
    g_t = g_of_node[col]
    p_t = (lr % P)[col]
    c_s = chunk[row].astype(np.int64)
    cid_s = cidx[row]

    # sort edges by (m_t, c_s, g_t, p_t)
    key_order = np.lexsort((p_t, g_t, c_s, m_t))
    sm, sc, sg, sp, scid = (m_t[key_order], c_s[key_order], g_t[key_order],
                            p_t[key_order], cid_s[key_order])
    # k within each (m, c, g, p) run
    cell = ((sm * NCHUNK + sc) * G + sg) * P + sp
    cell_counts = np.bincount(cell, minlength=NCORES * NCHUNK * G * P)
    cell_starts = np.concatenate([[0], np.cumsum(cell_counts)[:-1]])
    kk = np.arange(E, dtype=np.int64) - cell_starts[cell]
    assert (kk < K[sg, sc]).all(), "slot overflow: K schedule too small"

    # token index within (m, c): j = (colstart[g, c] + k) * P + p
    j_tok = (colstart[sg, sc] + kk) * P + sp

    TOKC = tokens_c  # per-chunk token count (same for all cores)
    SROW = TOKC // 16
    idx_arrays = []
    for m in range(NCORES):
        arrs = []
        for c in range(NCHUNK):
            arr = np.full(TOKC[c], ZIDX, dtype=np.int16)
            sel = (sm == m) & (sc == c)
            arr[j_tok[sel]] = scid[sel].astype(np.int16)
            # wrapped layout: idx16[j%16, j//16]
            arrs.append(np.ascontiguousarray(
                arr.reshape(-1, 16).T))  # [16, TOKC/16]
        idx_arrays.append(np.concatenate(arrs, axis=1))  # [16, sum(SROW)]

    # idx column offset of each instruction within the concatenated idx array
    chunk_col0 = np.concatenate([[0], np.cumsum(SROW)[:-1]])
    instr_idx_off = [int(chunk_col0[c] + col0 * (P // 16))
                     for (c, g0, ng, cols, col0) in instrs]

    # per-core node list in (g, p) order: node at lr
    nodes_of_core = np.full((NCORES, NPL), -1, dtype=np.int64)
    for m in range(NCORES):
        sel = np.nonzero(core == m)[0]
        nodes_of_core[m, lr[sel]] = sel

    # deg upload [P, G]: deg[p, g] ; dummies inf
    degp = np.full((NCORES, P, G), np.inf, dtype=np.float32)
    for m in range(NCORES):
        sel = nodes_of_core[m] >= 0
        lrs = np.nonzero(sel)[0]
        degp[m, lrs % P, lrs // P] = deg[nodes_of_core[m][lrs]] + 1.0

    return dict(
        NP=NP, G=G, NPL=NPL, NTBL=NTBL, CHROWS=CHROWS,
        K=K, instrs=instrs, runs=runs, instr_idx_off=instr_idx_off,
        idx_arrays=idx_arrays, idx_cols=int(SROW.sum()),
        nodes_of_core=nodes_of_core, degp=degp, tokens_c=TOKC,
    )


# --------------------------------------------------------------------------
# Device program
# --------------------------------------------------------------------------

def _build_program(plan, IN_CH, HID, OUT_CH):
    G, NPL, NTBL, CHROWS = plan["G"], plan["NPL"], plan["NTBL"], plan["CHROWS"]
    instrs, runs, idx_off = plan["instrs"], plan["runs"], plan["instr_idx_off"]
    NCH_IN = IN_CH // P
    IDXC = plan["idx_cols"]

    nc = bacc.Bacc(None, num_devices=NCORES, num_swdge_queues=4)

    xT_d = nc.dram_tensor("xT", [IN_CH, NPL], BF16, kind="ExternalInput")
    deg_d = nc.dram_tensor("deg", [P, G], F32, kind="ExternalInput")
    idx_d = nc.dram_tensor("idx", [P, IDXC], I16, kind="ExternalInput")
    w1_d = nc.dram_tensor("w1", [P, NCH_IN * HID], BF16, kind="ExternalInput")
    b1_d = nc.dram_tensor("b1", [1, HID], F32, kind="ExternalInput")
    w2_d = nc.dram_tensor("w2", [HID, OUT_CH], F32, kind="ExternalInput")
    b2_d = nc.dram_tensor("b2", [1, OUT_CH], F32, kind="ExternalInput")
    out_d = nc.dram_tensor("out", [P, G * OUT_CH], F32, kind="ExternalOutput")

    groups_all = [list(range(NCORES))]

    with tile.TileContext(nc) as tc:
        with (
            tc.tile_pool(name="const", bufs=1) as const,
            tc.tile_pool(name="persist", bufs=1) as persist,
            tc.tile_pool(name="idxp", bufs=8) as idxp,
            tc.tile_pool(name="gbp", bufs=8) as gbp,
            tc.tile_pool(name="vts", bufs=3) as vtsp,
            tc.tile_pool(name="dram", bufs=1, space="DRAM") as dram,
            tc.tile_pool(name="psH", bufs=4, space="PSUM") as psH,
            tc.tile_pool(name="psT", bufs=2, space="PSUM") as psT,
            tc.tile_pool(name="psO", bufs=2, space="PSUM") as psO,
        ):
            # ---- constants ----
            ident = const.tile([P, P], F32)
            make_identity(nc, ident[:])
            w1st = vtsp.tile([P, NCH_IN * HID], BF16, tag="vts")
            nc.sync.dma_start(w1st[:], w1_d[:])
            w1sb = const.tile([P, NCH_IN, HID], BF16)
            nc.vector.tensor_copy(
                w1sb[:], w1st[:].rearrange("k (c f) -> k c f", f=HID))
            w2st = vtsp.tile([HID, OUT_CH], F32, tag="vts2")
            nc.sync.dma_start(w2st[:], w2_d[:])
            w2sb = const.tile([HID, OUT_CH], F32)
            nc.vector.tensor_copy(w2sb[:], w2st[:])
            b1sb = const.tile([P, HID], F32)
            nc.sync.dma_start(b1sb[:], b1_d[:].to_broadcast([P, HID]))
            b2sb = const.tile([P, OUT_CH], F32)
            nc.sync.dma_start(b2sb[:], b2_d[:].to_broadcast([P, OUT_CH]))
            degsb = const.tile([P, G], F32)
            nc.sync.dma_start(degsb[:], deg_d[:])
            dis = const.tile([P, G], F32)
            nc.vector.reciprocal(dis[:], degsb[:])
            nc.scalar.activation(dis[:], dis[:],
                                 mybir.ActivationFunctionType.Sqrt)

            def dis_bc(F):
                return (dis[:].rearrange("p (g u) -> p g u", u=1)
                        .to_broadcast([P, G, F]))

            # ---- stage A: H = x @ W1  (lhsT = xT chunks) ----
            H = persist.tile([P, G * HID], F32, tag="H")
            with tc.tile_pool(name="xtp", bufs=1) as xtp:
                xts = []
                touch = xtp.tile([P, NCH_IN], BF16, tag="touch",
                                 name="touch")
                for c in range(NCH_IN):
                    xt_c = xtp.tile([P, NPL], BF16, tag=f"xt{c}",
                                    name=f"xt{c}")
                    nc.sync.dma_start(
                        xt_c[:], xT_d[c * P:(c + 1) * P, :])
                    # DVE touch: PE matmuls then wait on the DVE lane only
                    nc.vector.tensor_copy(touch[:, c:c + 1], xt_c[:, 0:1])
                    xts.append(xt_c)
                for g in range(G):
                    hps = psH.tile([P, HID], F32, tag="hps")
                    for c in range(NCH_IN):
                        nc.tensor.matmul(
                            hps[:],
                            lhsT=xts[c][:, g * P:(g + 1) * P],
                            rhs=w1sb[:, c, :],
                            start=(c == 0),
                            stop=(c == NCH_IN - 1),
                        )
                    nc.vector.tensor_copy(H[:, g * HID:(g + 1) * HID], hps[:])

            # ---- per-layer helpers ----
            def publish(tsrc, tag):
                """tsrc [P, G*HID] -> compact slice -> AllGather -> expanded
                256B-strided table. Returns the padded table dram tile."""
                compact_slice = dram.tile([NPL, HID], F32,
                                          tag=f"cs_{tag}", name=f"cs_{tag}")
                # node lr = g*128+p  ->  compact row lr
                nc.sync.dma_start(
                    compact_slice[:].rearrange("(g p) f -> p g f", p=P),
                    tsrc[:].rearrange("p (g f) -> p g f", f=HID),
                )
                compact = dram.tile([NTBL, HID], F32, tag=f"cf_{tag}",
                                    name=f"cf_{tag}", addr_space="Shared")
                nc.gpsimd.collective_compute(
                    "AllGather",
                    mybir.AluOpType.bypass,
                    ins=[compact_slice[:].opt()],
                    outs=[compact[:].opt()],
                    replica_groups=groups_all,
                )
                padded = dram.tile([NTBL, 64], F32, tag=f"pt_{tag}",
                                   name=f"pt_{tag}")
                for c in range(NCHUNK):
                    nc.sync.dma_start(
                        padded[c * CHROWS:(c + 1) * CHROWS, 0:HID],
                        compact[c * CHROWS:(c + 1) * CHROWS, :],
                    )
                return padded

            def gather_layer(padded, tag):
                """Run the padded-CSR gathers + reduces. Returns agg
                [P, G*HID] f32 (sum over in-neighbors of t-rows)."""
                partials = persist.tile([P, NCHUNK * G * HID], F32,
                                        tag="partials", name="partials")
                nc.vector.memset(partials[:], 0.0)
                for i, (c, g0, ng, cols, col0) in enumerate(instrs):
                    S = cols * (P // 16)
                    idx_t = idxp.tile([P, MAX_COLS * (P // 16)], I16,
                                      tag="idx")
                    nc.scalar.dma_start(
                        idx_t[:, 0:S],
                        idx_d[:, idx_off[i]:idx_off[i] + S],
                    )
                    gb = gbp.tile([P, MAX_COLS, HID], F32, tag="gb")
                    _raw_dma_gather(
                        nc.gpsimd,
                        gb[:, 0:cols, :],
                        padded[c * CHROWS:(c + 1) * CHROWS, 0:HID],
                        idx_t[:, 0:S],
                        cols * P,
                        queue_num=i % 4,
                    )
                    for (lc0, nng, kv, gg0) in runs[i]:
                        nc.vector.tensor_reduce(
                            out=partials[:].rearrange(
                                "p (cg f) -> p cg f", f=HID)[
                                :, c * G + gg0: c * G + gg0 + nng, :],
                            in_=gb[:, lc0:lc0 + nng * kv, :].rearrange(
                                "p (n k) f -> p n f k", k=kv),
                            axis=mybir.AxisListType.X,
                            op=mybir.AluOpType.add,
                        )
                agg = persist.tile([P, G * HID], F32, tag="agg",
                                   name="agg")
                nc.vector.tensor_reduce(
                    out=agg[:].rearrange("p (g f) -> p g f", f=HID),
                    in_=partials[:].rearrange("p (c g f) -> p g f c",
                                              c=NCHUNK, f=HID),
                    axis=mybir.AxisListType.X,
                    op=mybir.AluOpType.add,
                )
                return agg

            # ---- layer 1 ----
            t1 = persist.tile([P, G * HID], F32, tag="t1")
            nc.vector.tensor_tensor(
                out=t1[:].rearrange("p (g f) -> p g f", f=HID),
                in0=H[:].rearrange("p (g f) -> p g f", f=HID),
                in1=dis_bc(HID), op=mybir.AluOpType.mult)
            tbl1 = publish(t1, "t1")
            agg1 = gather_layer(tbl1, "l1")

            # out1 = dis*(agg1 + t1) + b1 ; h1 = relu ; t2 = dis*h1
            nc.vector.tensor_tensor(out=agg1[:], in0=agg1[:], in1=t1[:],
                                    op=mybir.AluOpType.add)
            nc.vector.tensor_tensor(
                out=agg1[:].rearrange("p (g f) -> p g f", f=HID),
                in0=agg1[:].rearrange("p (g f) -> p g f", f=HID),
                in1=dis_bc(HID), op=mybir.AluOpType.mult)
            nc.vector.tensor_tensor(
                out=agg1[:].rearrange("p (g f) -> p g f", f=HID),
                in0=agg1[:].rearrange("p (g f) -> p g f", f=HID),
                in1=b1sb[:].rearrange("p (u f) -> p u f", u=1)
                    .to_broadcast([P, G, HID]),
                op=mybir.AluOpType.add)
            nc.scalar.activation(agg1[:], agg1[:],
                                 mybir.ActivationFunctionType.Relu)
            t2 = persist.tile([P, G * HID], F32, tag="t2")
            nc.vector.tensor_tensor(
                out=t2[:].rearrange("p (g f) -> p g f", f=HID),
                in0=agg1[:].rearrange("p (g f) -> p g f", f=HID),
                in1=dis_bc(HID), op=mybir.AluOpType.mult)

            # ---- layer 2 ----
            tbl2 = publish(t2, "t2")
            agg2 = gather_layer(tbl2, "l2")

            nc.vector.tensor_tensor(out=agg2[:], in0=agg2[:], in1=t2[:],
                                    op=mybir.AluOpType.add)
            nc.vector.tensor_tensor(
                out=agg2[:].rearrange("p (g f) -> p g f", f=HID),
                in0=agg2[:].rearrange("p (g f) -> p g f", f=HID),
                in1=dis_bc(HID), op=mybir.AluOpType.mult)
            V = agg2

            # ---- O = V @ W2 + b2, log_softmax ----
            warm = psT.tile([P, P], F32, tag="pt")
            nc.tensor.transpose(warm[:], ident[:], ident[:])
            O = persist.tile([P, G * OUT_CH], F32, tag="O")
            for g in range(G):
                vt_ps = psT.tile([P, P], F32, tag="pt")
                nc.tensor.transpose(
                    vt_ps[:HID, :], V[:, g * HID:(g + 1) * HID], ident[:])
                vt_sb = vtsp.tile([HID, P], F32, tag="vts3")
                nc.vector.tensor_copy(vt_sb[:], vt_ps[:HID, :])
                ops = psO.tile([P, OUT_CH], F32, tag="ops")
                nc.tensor.matmul(ops[:], lhsT=vt_sb[:], rhs=w2sb[:],
                                 start=True, stop=True)
                nc.vector.tensor_copy(O[:, g * OUT_CH:(g + 1) * OUT_CH],
                                      ops[:])

            O3 = O[:].rearrange("p (g f) -> p g f", f=OUT_CH)
            nc.vector.tensor_tensor(
                out=O3, in0=O3,
                in1=b2sb[:].rearrange("p (u f) -> p u f", u=1)
                    .to_broadcast([P, G, OUT_CH]),
                op=mybir.AluOpType.add)
            mx = const.tile([P, G], F32)
            nc.vector.tensor_reduce(out=mx[:], in_=O3,
                                    axis=mybir.AxisListType.X,
                                    op=mybir.AluOpType.max)
            nc.vector.tensor_tensor(
                out=O3, in0=O3,
                in1=mx[:].rearrange("p (g u) -> p g u", u=1)
                    .to_broadcast([P, G, OUT_CH]),
                op=mybir.AluOpType.subtract)
            nc.scalar.activation(O[:], O[:],
                                 mybir.ActivationFunctionType.Exp)
            sm = const.tile([P, G], F32)
            nc.vector.tensor_reduce(
                out=sm[:],
                in_=O[:].rearrange("p (g f) -> p g f", f=OUT_CH),
                axis=mybir.AxisListType.X, op=mybir.AluOpType.add)
            nc.scalar.activation(sm[:], sm[:],
                                 mybir.ActivationFunctionType.Ln)
            # back to logits: ln(exp(x)) = x to ~1e-7; then subtract ln(sum)
            nc.scalar.activation(O[:], O[:],
                                 mybir.ActivationFunctionType.Ln)
            nc.vector.tensor_tensor(
                out=O3, in0=O3,
                in1=sm[:].rearrange("p (g u) -> p g u", u=1)
                    .to_broadcast([P, G, OUT_CH]),
                op=mybir.AluOpType.subtract)
            nc.sync.dma_start(out_d[:], O[:])

    nc.finalize()
    return nc


# --------------------------------------------------------------------------
# Entry point
# --------------------------------------------------------------------------

def kernel(x, W1, b1, W2, b2, edge_index, _trace=False):
    import ml_dtypes

    x = np.asarray(x, dtype=np.float32)
    W1 = np.asarray(W1, dtype=np.float32)
    b1 = np.asarray(b1, dtype=np.float32)
    W2 = np.asarray(W2, dtype=np.float32)
    b2 = np.asarray(b2, dtype=np.float32)
    edge_index = np.asarray(edge_index)

    N, IN_CH = x.shape
    HID = W1.shape[1]
    OUT_CH = W2.shape[1]

    plan = _plan(edge_index, N)
    kernel._plan = plan
    G, NPL = plan["G"], plan["NPL"]
    NCH_IN = IN_CH // P

    nc = _build_program(plan, IN_CH, HID, OUT_CH)

    # per-core inputs
    w1p = np.ascontiguousarray(
        W1.reshape(NCH_IN, P, HID).transpose(1, 0, 2).reshape(P, NCH_IN * HID)
    ).astype(ml_dtypes.bfloat16)
    maps = []
    for m in range(NCORES):
        nodes = plan["nodes_of_core"][m]
        xTm = np.zeros((IN_CH, NPL), dtype=ml_dtypes.bfloat16)
        sel = nodes >= 0
        xTm[:, sel] = x[nodes[sel]].T.astype(ml_dtypes.bfloat16)
        maps.append(dict(
            xT=xTm,
            deg=np.ascontiguousarray(plan["degp"][m]),
            idx=np.ascontiguousarray(np.tile(plan["idx_arrays"][m], (8, 1))),
            w1=w1p,
            b1=b1.reshape(1, HID),
            w2=W2,
            b2=b2.reshape(1, OUT_CH),
        ))

    r = run_bass_kernel_spmd(nc, maps, core_ids=list(range(NCORES)),
                             trace=_trace)
    kernel._exec_ns = r.exec_time_ns or 0

    out = np.empty((N, OUT_CH), dtype=np.float32)
    for m in range(NCORES):
        Om = np.asarray(r.results[m]["out"]).reshape(P, G, OUT_CH)
        nodes = plan["nodes_of_core"][m]
        sel = nodes >= 0
        lrs = np.nonzero(sel)[0]
        out[nodes[lrs]] = Om[lrs % P, lrs // P, :]
    return out


# revision 12
# speedup vs baseline: 1.3426x; 1.0016x over previous
"""Two-layer GCN forward (GCNConv -> relu -> GCNConv -> log_softmax) on 8
Trainium2 NeuronCores, single NEFF.

Design (pull-based aggregation at width 16):
  - Nodes are degree-sorted and assigned to cores so every core owns NP nodes
    with an identical degree profile (group g on every core holds the nodes of
    global rank window [1024g, 1024(g+1)) -- required because the SPMD padded
    CSR schedule K[g][chunk] is shared across cores).
  - The "message table" t = dis * h (16 f32 per node) is AllGathered compactly
    ([NTBL, 16] f32) then locally expanded to a 256B-strided layout
    ([NTBL, 64] f32, row i = node i) so the bulk SWDGE gather instruction
    (InstDMAGatherAnt, 64B payload, 256B row stride, int16 idx) can fetch
    per-edge rows. int16 limits one gather to 32k rows, so the table is split
    into 4 chunks of 2*NPL rows (chunk c = cores 2c,2c+1's slices); a host-side
    balancer assigns nodes to core-pairs to equalize each target's in-neighbor
    count per chunk, shrinking the padded-CSR K.
  - Per (group, chunk) the padded CSR has K[g][c] slots per target; gathers are
    issued in <=8192-token instructions round-robin over the 4 SWDGE queues
    (descriptor generation on the Q7 cores is the bottleneck; 4 queues run it
    in parallel). Pad slots point at a dummy node row which is always zero
    (dummy nodes get dis = 0). A strided DVE tensor_reduce sums the K slots.
  - norm factorizes: norm[e] = dis[src]*dis[dst], dis = rsqrt(deg+1), so
    out = dis * (gathered_sum + t_self) + bias per layer; layer 2 aggregates at
    width 16 and applies W2 afterwards (aggregation commutes with the output
    projection).

kernel(**inputs) takes full unsharded inputs, returns the full [N, 40] output.
"""

import sys

import numpy as np

try:
    import concourse.bass as bass
except ImportError:  # pragma: no cover
    sys.path.insert(0, "/opt/trn_rl_repo")
    import concourse.bass as bass

import concourse.bacc as bacc
import concourse.tile as tile
from concourse import mybir
from concourse.bass_utils import run_bass_kernel_spmd
from concourse.masks import make_identity

F32 = mybir.dt.float32
BF16 = mybir.dt.bfloat16
I16 = mybir.dt.int16

NCORES = 8
P = 128
NCHUNK = 4
MAX_TOK = 8192  # tokens per gather instruction (HW-validated bound)
MAX_COLS = MAX_TOK // P


def _raw_dma_gather(g, out_ap, in_ap, idxs_ap, num_idxs, queue_num):
    """dma_gather with 64B payload (elem_size=16 f32) at 256B row stride.

    Mirrors bass's dma_gather lowering minus its 256B elem_size assert (the
    ucode only requires the row *stride* and the base address to be 256B
    aligned; 64B payloads verified on HW)."""
    from concourse.bass import MemorySpace

    assert idxs_ap.dtype == I16
    assert in_ap.space == MemorySpace.DRAM
    assert out_ap.space == MemorySpace.SBUF
    assert in_ap.ap[-1][1] == 16 and out_ap.ap[-1][1] == 16
    assert in_ap.ap[0][0] == 64  # elem_step: 64 f32 = 256B
    _in_ap = g.lower_ap_dma(in_ap, for_custom_bir_dma=True)
    _idxs_ap = g.lower_ap(idxs_ap)
    _out_ap = g.lower_ap(out_ap)
    return g.add_instruction(
        mybir.InstDMAGatherAnt(
            name=g.bass.get_next_instruction_name(),
            ins=[*_in_ap, _idxs_ap, g.lower_val_access(g.to_reg(num_idxs))],
            outs=[_out_ap],
            transpose=False,
            num_idxs=num_idxs,
            elem_size=16,
            stride_bytes_256=1,
            gen_mode=0,
            single_packet=False,
            queue_num=queue_num,
            sbuf_tokens_per_rank=0,
            sbuf_free_dim_per_rank=0,
            sbuf_free_dim_pad_per_rank=0,
            sbuf_byte_offset=0,
        )
    )


# --------------------------------------------------------------------------
# Host planner
# --------------------------------------------------------------------------

def _balance_chunks(row, col, deg, order, N, W, npasses=3, seed=0):
    """Assign each node to one of 4 chunks (= core pairs), W//4 per W-rank
    window, minimizing per-(target, chunk) in-neighbor overflow vs the
    per-target quota ceil(deg/4). Returns chunk[node] in 0..3."""
    quota_t = np.ceil(deg / NCHUNK).astype(np.int32)
    rng = np.random.default_rng(seed)
    chunk = np.empty(N, dtype=np.int8)
    chunk[order] = (np.arange(N) % NCHUNK).astype(np.int8)

    k = np.zeros((N, NCHUNK), dtype=np.int32)
    np.add.at(k, (col, chunk[row]), 1)

    src_sort = np.argsort(row, kind="stable")
    counts = np.bincount(row, minlength=N)
    starts = np.concatenate([[0], np.cumsum(counts)])
    NW = -(-N // W)

    for _ in range(npasses):
        for w in range(NW):
            ranks = np.arange(w * W, min((w + 1) * W, N))
            nodes = order[ranks]
            lo = starts[nodes]
            hi = starts[nodes + 1]
            lens = hi - lo
            if lens.sum() == 0:
                continue
            idx_edges = np.concatenate(
                [src_sort[a:b] for a, b in zip(lo, hi)])
            esrc = row[idx_edges]
            etgt = col[idx_edges]
            np.add.at(k, (etgt, chunk[esrc]), -1)
            pen = np.maximum(
                0, k[etgt] - quota_t[etgt][:, None] + 1).astype(np.float32)
            node_of_edge = np.repeat(np.arange(len(nodes)), lens)
            score = np.zeros((len(nodes), NCHUNK), dtype=np.float32)
            np.add.at(score, node_of_edge, pen)
            score += rng.random(score.shape).astype(np.float32) * 1e-3
            q = np.full(NCHUNK, len(nodes) // NCHUNK, dtype=np.int64)
            q[: len(nodes) - q.sum()] += 1
            srt = np.argsort(score, axis=1)
            regret = (score[np.arange(len(nodes)), srt[:, 1]]
                      - score[np.arange(len(nodes)), srt[:, 0]])
            for i in np.argsort(-regret):
                for c in srt[i]:
                    if q[c] > 0:
                        chunk[nodes[i]] = c
                        q[c] -= 1
                        break
            np.add.at(k, (etgt, chunk[esrc]), 1)
    return chunk, k


def _plan(edge_index, N):
    row = np.asarray(edge_index[0], dtype=np.int64)
    col = np.asarray(edge_index[1], dtype=np.int64)
    E = row.size

    NP = N // NCORES               # real nodes per core
    G = -(-NP // P)                # groups per core
    NPL = G * P                    # padded nodes per core
    NTBL = NCORES * NPL            # table rows
    CHROWS = 2 * NPL               # rows per chunk (= core pair)
    W = NCORES * P                 # rank window feeding one group (1024)

    deg = np.bincount(col, minlength=N)
    order = np.argsort(-deg, kind="stable")

    chunk, k_tc = _balance_chunks(row, col, deg, order, N, W)

    # core assignment: within chunk c, nodes by rank alternate cores 2c/2c+1
    rank = np.empty(N, dtype=np.int64)
    rank[order] = np.arange(N)
    core = np.empty(N, dtype=np.int64)
    lr = np.empty(N, dtype=np.int64)
    for c in range(NCHUNK):
        nodes_c = np.nonzero(chunk == c)[0]
        o = np.argsort(rank[nodes_c], kind="stable")
        nodes_c = nodes_c[o]
        core[nodes_c] = 2 * c + (np.arange(nodes_c.size) % 2)
        lr[nodes_c] = np.arange(nodes_c.size) // 2

    assert lr.max() < NP, f"core overflow: {lr.max()} >= {NP}"
    tid = core * NPL + lr
    cidx = (tid % CHROWS).astype(np.int64)
    ZIDX = NP  # dummy row (lr = NP of the even core) -> always zero
    assert NP < NPL, "need at least one dummy row per core"

    # K schedule: K[g][c] = max in-neighbor-count-from-chunk-c over targets in
    # group g (across all cores; group g <-> rank window g)
    g_of_node = lr // P
    K = np.zeros((G, NCHUNK), dtype=np.int64)
    for c in range(NCHUNK):
        kk = np.zeros(G, dtype=np.int64)
        np.maximum.at(kk, g_of_node, k_tc[:, c])
        K[:, c] = kk
    K = np.maximum(K, 0)

    # instruction packing: per chunk, consecutive groups with sum(K) <= MAX_COLS
    instrs = []  # (c, g0, n_groups, cols, col0) ; col0 = within-chunk col offset
    colstart = np.zeros((G, NCHUNK), dtype=np.int64)
    tokens_c = np.zeros(NCHUNK, dtype=np.int64)
    for c in range(NCHUNK):
        colstart[:, c] = np.concatenate([[0], np.cumsum(K[:, c])[:-1]])
        tokens_c[c] = K[:, c].sum() * P
        g0 = 0
        while g0 < G:
            cols = 0
            g1 = g0
            while g1 < G and cols + K[g1, c] <= MAX_COLS:
                cols += K[g1, c]
                g1 += 1
            if cols == 0:  # single group exceeding MAX_COLS (can't happen: K<=64)
                raise AssertionError("group K exceeds MAX_COLS")
            instrs.append((c, g0, g1 - g0, int(cols), int(colstart[g0, c])))
            g0 = g1

    # K-runs per instruction for the reduce stage
    runs = []  # per instr: list of (local_col0, n_groups, K, g0)
    for (c, g0, ng, cols, col0) in instrs:
        rr = []
        i = g0
        while i < g0 + ng:
            j = i
            while j < g0 + ng and K[j, c] == K[i, c]:
                j += 1
            if K[i, c] > 0:
                rr.append((int(colstart[i, c] - col0), j - i, int(K[i, c]), i))
            i = j
        runs.append(rr)

    # ---- per-core slot tables (idx arrays) ----
    # edge slot: target (m, g, p), source chunk c, k-th occurrence
    m_t = core[col]
    g_t = g_of_node[col]
    p_t = (lr % P)[col]
    c_s = chunk[row].astype(np.int64)
    cid_s = cidx[row]

    # sort edges by (m_t, c_s, g_t, p_t)
    key_order = np.lexsort((p_t, g_t, c_s, m_t))
    sm, sc, sg, sp, scid = (m_t[key_order], c_s[key_order], g_t[key_order],
                            p_t[key_order], cid_s[key_order])
    # k within each (m, c, g, p) run
    cell = ((sm * NCHUNK + sc) * G + sg) * P + sp
    cell_counts = np.bincount(cell, minlength=NCORES * NCHUNK * G * P)
    cell_starts = np.concatenate([[0], np.cumsum(cell_counts)[:-1]])
    kk = np.arange(E, dtype=np.int64) - cell_starts[cell]
    assert (kk < K[sg, sc]).all(), "slot overflow: K schedule too small"

    # token index within (m, c): j = (colstart[g, c] + k) * P + p
    j_tok = (colstart[sg, sc] + kk) * P + sp

    TOKC = tokens_c  # per-chunk token count (same for all cores)
    SROW = TOKC // 16
    idx_arrays = []
    for m in range(NCORES):
        arrs = []
        for c in range(NCHUNK):
            arr = np.full(TOKC[c], ZIDX, dtype=np.int16)
            sel = (sm == m) & (sc == c)
            arr[j_tok[sel]] = scid[sel].astype(np.int16)
            # wrapped layout: idx16[j%16, j//16]
            arrs.append(np.ascontiguousarray(
                arr.reshape(-1, 16).T))  # [16, TOKC/16]
        idx_arrays.append(np.concatenate(arrs, axis=1))  # [16, sum(SROW)]

    # idx column offset of each instruction within the concatenated idx array
    chunk_col0 = np.concatenate([[0], np.cumsum(SROW)[:-1]])
    instr_idx_off = [int(chunk_col0[c] + col0 * (P // 16))
                     for (c, g0, ng, cols, col0) in instrs]

    # per-core node list in (g, p) order: node at lr
    nodes_of_core = np.full((NCORES, NPL), -1, dtype=np.int64)
    for m in range(NCORES):
        sel = np.nonzero(core == m)[0]
        nodes_of_core[m, lr[sel]] = sel

    # deg upload [P, G]: deg[p, g] ; dummies inf
    degp = np.full((NCORES, P, G), np.inf, dtype=np.float32)
    for m in range(NCORES):
        sel = nodes_of_core[m] >= 0
        lrs = np.nonzero(sel)[0]
        degp[m, lrs % P, lrs // P] = deg[nodes_of_core[m][lrs]] + 1.0

    return dict(
        NP=NP, G=G, NPL=NPL, NTBL=NTBL, CHROWS=CHROWS,
        K=K, instrs=instrs, runs=runs, instr_idx_off=instr_idx_off,
        idx_arrays=idx_arrays, idx_cols=int(SROW.sum()),
        nodes_of_core=nodes_of_core, degp=degp, tokens_c=TOKC,
    )


# --------------------------------------------------------------------------
# Device program
# --------------------------------------------------------------------------

def _build_program(plan, IN_CH, HID, OUT_CH):
    G, NPL, NTBL, CHROWS = plan["G"], plan["NPL"], plan["NTBL"], plan["CHROWS"]
    instrs, runs, idx_off = plan["instrs"], plan["runs"], plan["instr_idx_off"]
    NCH_IN = IN_CH // P
    IDXC = plan["idx_cols"]

    nc = bacc.Bacc(None, num_devices=NCORES, num_swdge_queues=4)

    xT_d = nc.dram_tensor("xT", [IN_CH, NPL], BF16, kind="ExternalInput")
    deg_d = nc.dram_tensor("deg", [P, G], F32, kind="ExternalInput")
    idx_d = nc.dram_tensor("idx", [P, IDXC], I16, kind="ExternalInput")
    w1_d = nc.dram_tensor("w1", [P, NCH_IN * HID], BF16, kind="ExternalInput")
    b1_d = nc.dram_tensor("b1", [1, HID], F32, kind="ExternalInput")
    w2_d = nc.dram_tensor("w2", [HID, OUT_CH], F32, kind="ExternalInput")
    b2_d = nc.dram_tensor("b2", [1, OUT_CH], F32, kind="ExternalInput")
    out_d = nc.dram_tensor("out", [P, G * OUT_CH], F32, kind="ExternalOutput")

    groups_all = [list(range(NCORES))]

    with tile.TileContext(nc) as tc:
        with (
            tc.tile_pool(name="const", bufs=1) as const,
            tc.tile_pool(name="persist", bufs=1) as persist,
            tc.tile_pool(name="idxp", bufs=8) as idxp,
            tc.tile_pool(name="gbp", bufs=8) as gbp,
            tc.tile_pool(name="vts", bufs=3) as vtsp,
            tc.tile_pool(name="dram", bufs=1, space="DRAM") as dram,
            tc.tile_pool(name="psH", bufs=4, space="PSUM") as psH,
            tc.tile_pool(name="psT", bufs=2, space="PSUM") as psT,
            tc.tile_pool(name="psO", bufs=2, space="PSUM") as psO,
        ):
            # ---- constants ----
            ident = const.tile([P, P], F32)
            make_identity(nc, ident[:])
            w1st = vtsp.tile([P, NCH_IN * HID], BF16, tag="vts")
            nc.sync.dma_start(w1st[:], w1_d[:])
            w1sb = const.tile([P, NCH_IN, HID], BF16)
            nc.vector.tensor_copy(
                w1sb[:], w1st[:].rearrange("k (c f) -> k c f", f=HID))
            w2st = vtsp.tile([HID, OUT_CH], F32, tag="vts2")
            nc.sync.dma_start(w2st[:], w2_d[:])
            w2sb = const.tile([HID, OUT_CH], F32)
            nc.vector.tensor_copy(w2sb[:], w2st[:])
            b1sb = const.tile([P, HID], F32)
            nc.sync.dma_start(b1sb[:], b1_d[:].to_broadcast([P, HID]))
            b2sb = const.tile([P, OUT_CH], F32)
            nc.sync.dma_start(b2sb[:], b2_d[:].to_broadcast([P, OUT_CH]))
            degsb = const.tile([P, G], F32)
            nc.sync.dma_start(degsb[:], deg_d[:])
            dis = const.tile([P, G], F32)
            nc.vector.reciprocal(dis[:], degsb[:])
            nc.scalar.activation(dis[:], dis[:],
                                 mybir.ActivationFunctionType.Sqrt)

            def dis_bc(F):
                return (dis[:].rearrange("p (g u) -> p g u", u=1)
                        .to_broadcast([P, G, F]))

            # ---- stage A: H = x @ W1  (lhsT = xT chunks) ----
            H = persist.tile([P, G * HID], F32, tag="H")
            with tc.tile_pool(name="xtp", bufs=1) as xtp:
                xts = []
                touch = xtp.tile([P, NCH_IN], BF16, tag="touch",
                                 name="touch")
                for c in range(NCH_IN):
                    xt_c = xtp.tile([P, NPL], BF16, tag=f"xt{c}",
                                    name=f"xt{c}")
                    nc.sync.dma_start(
                        xt_c[:], xT_d[c * P:(c + 1) * P, :])
                    # DVE touch: PE matmuls then wait on the DVE lane only
                    nc.vector.tensor_copy(touch[:, c:c + 1], xt_c[:, 0:1])
                    xts.append(xt_c)
                for g in range(G):
                    hps = psH.tile([P, HID], F32, tag="hps")
                    for c in range(NCH_IN):
                        nc.tensor.matmul(
                            hps[:],
                            lhsT=xts[c][:, g * P:(g + 1) * P],
                            rhs=w1sb[:, c, :],
                            start=(c == 0),
                            stop=(c == NCH_IN - 1),
                        )
                    nc.vector.tensor_copy(H[:, g * HID:(g + 1) * HID], hps[:])

            # ---- per-layer helpers ----
            def publish(tsrc, tag):
                """tsrc [P, G*HID] -> compact slice -> AllGather -> expanded
                256B-strided table. Returns the padded table dram tile."""
                compact_slice = dram.tile([NPL, HID], F32,
                                          tag=f"cs_{tag}", name=f"cs_{tag}")
                # node lr = g*128+p  ->  compact row lr
                nc.sync.dma_start(
                    compact_slice[:].rearrange("(g p) f -> p g f", p=P),
                    tsrc[:].rearrange("p (g f) -> p g f", f=HID),
                )
                compact = dram.tile([NTBL, HID], F32, tag=f"cf_{tag}",
                                    name=f"cf_{tag}", addr_space="Shared")
                nc.gpsimd.collective_compute(
                    "AllGather",
                    mybir.AluOpType.bypass,
                    ins=[compact_slice[:].opt()],
                    outs=[compact[:].opt()],
                    replica_groups=groups_all,
                )
                padded = []
                for c in range(NCHUNK):
                    pc = dram.tile([CHROWS, 64], F32, tag=f"pt_{tag}{c}",
                                   name=f"pt_{tag}{c}")
                    nc.sync.dma_start(
                        pc[:, 0:HID],
                        compact[c * CHROWS:(c + 1) * CHROWS, :],
                    )
                    padded.append(pc)
                return padded

            def gather_layer(padded, tag):
                """Run the padded-CSR gathers + reduces. Returns agg
                [P, G*HID] f32 (sum over in-neighbors of t-rows)."""
                partials = persist.tile([P, NCHUNK * G * HID], F32,
                                        tag="partials", name="partials")
                nc.vector.memset(partials[:], 0.0)
                for i, (c, g0, ng, cols, col0) in enumerate(instrs):
                    S = cols * (P // 16)
                    idx_t = idxp.tile([P, MAX_COLS * (P // 16)], I16,
                                      tag="idx")
                    nc.scalar.dma_start(
                        idx_t[:, 0:S],
                        idx_d[:, idx_off[i]:idx_off[i] + S],
                    )
                    gb = gbp.tile([P, MAX_COLS, HID], F32, tag="gb")
                    _raw_dma_gather(
                        nc.gpsimd,
                        gb[:, 0:cols, :],
                        padded[c][:, 0:HID],
                        idx_t[:, 0:S],
                        cols * P,
                        queue_num=i % 4,
                    )
                    for (lc0, nng, kv, gg0) in runs[i]:
                        nc.vector.tensor_reduce(
                            out=partials[:].rearrange(
                                "p (cg f) -> p cg f", f=HID)[
                                :, c * G + gg0: c * G + gg0 + nng, :],
                            in_=gb[:, lc0:lc0 + nng * kv, :].rearrange(
                                "p (n k) f -> p n f k", k=kv),
                            axis=mybir.AxisListType.X,
                            op=mybir.AluOpType.add,
                        )
                agg = persist.tile([P, G * HID], F32, tag="agg",
                                   name="agg")
                nc.vector.tensor_reduce(
                    out=agg[:].rearrange("p (g f) -> p g f", f=HID),
                    in_=partials[:].rearrange("p (c g f) -> p g f c",
                                              c=NCHUNK, f=HID),
                    axis=mybir.AxisListType.X,
                    op=mybir.AluOpType.add,
                )
                return agg

            # ---- layer 1 ----
            t1 = persist.tile([P, G * HID], F32, tag="t1")
            nc.vector.tensor_tensor(
                out=t1[:].rearrange("p (g f) -> p g f", f=HID),
                in0=H[:].rearrange("p (g f) -> p g f", f=HID),
                in1=dis_bc(HID), op=mybir.AluOpType.mult)
            tbl1 = publish(t1, "t1")
            agg1 = gather_layer(tbl1, "l1")

            # out1 = dis*(agg1 + t1) + b1 ; h1 = relu ; t2 = dis*h1
            nc.vector.tensor_tensor(out=agg1[:], in0=agg1[:], in1=t1[:],
                                    op=mybir.AluOpType.add)
            nc.vector.tensor_tensor(
                out=agg1[:].rearrange("p (g f) -> p g f", f=HID),
                in0=agg1[:].rearrange("p (g f) -> p g f", f=HID),
                in1=dis_bc(HID), op=mybir.AluOpType.mult)
            nc.vector.tensor_tensor(
                out=agg1[:].rearrange("p (g f) -> p g f", f=HID),
                in0=agg1[:].rearrange("p (g f) -> p g f", f=HID),
                in1=b1sb[:].rearrange("p (u f) -> p u f", u=1)
                    .to_broadcast([P, G, HID]),
                op=mybir.AluOpType.add)
            nc.scalar.activation(agg1[:], agg1[:],
                                 mybir.ActivationFunctionType.Relu)
            t2 = persist.tile([P, G * HID], F32, tag="t2")
            nc.vector.tensor_tensor(
                out=t2[:].rearrange("p (g f) -> p g f", f=HID),
                in0=agg1[:].rearrange("p (g f) -> p g f", f=HID),
                in1=dis_bc(HID), op=mybir.AluOpType.mult)

            # ---- layer 2 ----
            tbl2 = publish(t2, "t2")
            agg2 = gather_layer(tbl2, "l2")

            nc.vector.tensor_tensor(out=agg2[:], in0=agg2[:], in1=t2[:],
                                    op=mybir.AluOpType.add)
            nc.vector.tensor_tensor(
                out=agg2[:].rearrange("p (g f) -> p g f", f=HID),
                in0=agg2[:].rearrange("p (g f) -> p g f", f=HID),
                in1=dis_bc(HID), op=mybir.AluOpType.mult)
            V = agg2

            # ---- O = V @ W2 + b2, log_softmax ----
            warm = psT.tile([P, P], F32, tag="pt")
            nc.tensor.transpose(warm[:], ident[:], ident[:])
            O = persist.tile([P, G * OUT_CH], F32, tag="O")
            for g in range(G):
                vt_ps = psT.tile([P, P], F32, tag="pt")
                nc.tensor.transpose(
                    vt_ps[:HID, :], V[:, g * HID:(g + 1) * HID], ident[:])
                vt_sb = vtsp.tile([HID, P], F32, tag="vts3")
                nc.vector.tensor_copy(vt_sb[:], vt_ps[:HID, :])
                ops = psO.tile([P, OUT_CH], F32, tag="ops")
                nc.tensor.matmul(ops[:], lhsT=vt_sb[:], rhs=w2sb[:],
                                 start=True, stop=True)
                nc.vector.tensor_copy(O[:, g * OUT_CH:(g + 1) * OUT_CH],
                                      ops[:])

            O3 = O[:].rearrange("p (g f) -> p g f", f=OUT_CH)
            nc.vector.tensor_tensor(
                out=O3, in0=O3,
                in1=b2sb[:].rearrange("p (u f) -> p u f", u=1)
                    .to_broadcast([P, G, OUT_CH]),
                op=mybir.AluOpType.add)
            mx = const.tile([P, G], F32)
            nc.vector.tensor_reduce(out=mx[:], in_=O3,
                                    axis=mybir.AxisListType.X,
                                    op=mybir.AluOpType.max)
            nc.vector.tensor_tensor(
                out=O3, in0=O3,
                in1=mx[:].rearrange("p (g u) -> p g u", u=1)
                    .to_broadcast([P, G, OUT_CH]),
                op=mybir.AluOpType.subtract)
            nc.scalar.activation(O[:], O[:],
                                 mybir.ActivationFunctionType.Exp)
            sm = const.tile([P, G], F32)
            nc.vector.tensor_reduce(
                out=sm[:],
                in_=O[:].rearrange("p (g f) -> p g f", f=OUT_CH),
                axis=mybir.AxisListType.X, op=mybir.AluOpType.add)
            nc.scalar.activation(sm[:], sm[:],
                                 mybir.ActivationFunctionType.Ln)
            # back to logits: ln(exp(x)) = x to ~1e-7; then subtract ln(sum)
            nc.scalar.activation(O[:], O[:],
                                 mybir.ActivationFunctionType.Ln)
            nc.vector.tensor_tensor(
                out=O3, in0=O3,
                in1=sm[:].rearrange("p (g u) -> p g u", u=1)
                    .to_broadcast([P, G, OUT_CH]),
                op=mybir.AluOpType.subtract)
            nc.sync.dma_start(out_d[:], O[:])

    nc.finalize()
    return nc


# --------------------------------------------------------------------------
# Entry point
# --------------------------------------------------------------------------

def kernel(x, W1, b1, W2, b2, edge_index, _trace=False):
    import ml_dtypes

    x = np.asarray(x, dtype=np.float32)
    W1 = np.asarray(W1, dtype=np.float32)
    b1 = np.asarray(b1, dtype=np.float32)
    W2 = np.asarray(W2, dtype=np.float32)
    b2 = np.asarray(b2, dtype=np.float32)
    edge_index = np.asarray(edge_index)

    N, IN_CH = x.shape
    HID = W1.shape[1]
    OUT_CH = W2.shape[1]

    plan = _plan(edge_index, N)
    kernel._plan = plan
    G, NPL = plan["G"], plan["NPL"]
    NCH_IN = IN_CH // P

    nc = _build_program(plan, IN_CH, HID, OUT_CH)

    # per-core inputs
    w1p = np.ascontiguousarray(
        W1.reshape(NCH_IN, P, HID).transpose(1, 0, 2).reshape(P, NCH_IN * HID)
    ).astype(ml_dtypes.bfloat16)
    maps = []
    for m in range(NCORES):
        nodes = plan["nodes_of_core"][m]
        xTm = np.zeros((IN_CH, NPL), dtype=ml_dtypes.bfloat16)
        sel = nodes >= 0
        xTm[:, sel] = x[nodes[sel]].T.astype(ml_dtypes.bfloat16)
        maps.append(dict(
            xT=xTm,
            deg=np.ascontiguousarray(plan["degp"][m]),
            idx=np.ascontiguousarray(np.tile(plan["idx_arrays"][m], (8, 1))),
            w1=w1p,
            b1=b1.reshape(1, HID),
            w2=W2,
            b2=b2.reshape(1, OUT_CH),
        ))

    r = run_bass_kernel_spmd(nc, maps, core_ids=list(range(NCORES)),
                             trace=_trace)
    kernel._exec_ns = r.exec_time_ns or 0

    out = np.empty((N, OUT_CH), dtype=np.float32)
    for m in range(NCORES):
        Om = np.asarray(r.results[m]["out"]).reshape(P, G, OUT_CH)
        nodes = plan["nodes_of_core"][m]
        sel = nodes >= 0
        lrs = np.nonzero(sel)[0]
        out[nodes[lrs]] = Om[lrs % P, lrs // P, :]
    return out
